# revision 1
# baseline (speedup 1.0000x reference)
"""Trainium2 Bass kernel for nn_Cross_Fusion_1047972020964.

Math (validated against the reference): complex_relu is the identity map on
nonzero inputs, so IDFT_l(DFT_l(x)*s + bias) collapses to
    out[b, t, :] = s[b, :] * x[b, t, :]          (t <  len_x[b], t != 0)
    out[b, 0, :] = s[b, :] * x[b, 0, :] + bias[b, :]
    out[b, t, :] = 0                             (t >= len_x[b])
with s = 1 + (W1o + W2o)/2 and bias = (B1o + B2o)/2 real, from 4 small MLPs
(exact-erf GELU) on c1 = sum_t y / len_y and c2 = sum_t z / len_z.

The CoreSim cost model is latency-dominated (DMA transfers overlap freely
across the SP/ACT/Pool issue rings; each DMA costs ~500ns issue + 650ns DGE
+ transfer + 900ns completion-sem), so the design minimizes the z/y ->
c -> MLP -> s -> elementwise -> store chain:
  - y/z/weights in fp8(e4m3), x in fp16, out in fp16 (error budget ~1e-3
    vs the 2e-2 gate).
  - x and out are d-major [d, t] so s (a per-d column) is a per-partition
    scalar: no broadcast matmuls needed.
  - c sums run on the PE (lhsT = y tile, rhs = ones) unscaled; the 1/len
    lands later as the gelu's per-partition scale AP, so lens are only
    needed just before the gelu, letting them ride in the weights DMA.
  - everything is split per sample so sample 0's pipeline starts on z0/y0
    without waiting for z1/y1.
"""

import os
import sys

import numpy as np

for _p in ("/opt/trn_rl_repo", "/root/.axon_site/_ro/trn_rl_repo"):
    if os.path.isdir(_p) and _p not in sys.path:
        sys.path.append(_p)

import ml_dtypes

import concourse.bass as bass
import concourse.tile as tile
from concourse import bacc, mybir
from concourse.alu_op_type import AluOpType as OP

B, L, D, H = 16, 1024, 128, 256
NCORES = 8
PB = B // NCORES          # samples per core
NT = L // 128             # 128-row tiles per sample (t-major y/z)
F32 = mybir.dt.float32
F16 = mybir.dt.float16
F8 = mybir.dt.float8e4
I32 = mybir.dt.int32
AF = mybir.ActivationFunctionType
NP8 = ml_dtypes.float8_e4m3
NETS = ("W1", "B1", "W2", "B2")  # nets 0,1 read c1 (y); nets 2,3 read c2 (z)

# packed weights+lens layout: cols [0:24) = lens6 int32 bytes,
# [24:1048) = l1 weights, [1048:2072) = l2 weights
WL = 24
W1OFF = WL
W2OFF = WL + 1024
WCOLS = WL + 2048


def build_nc(act=AF.Gelu):
    nc = bacc.Bacc("TRN2", target_bir_lowering=False, debug=False)

    xd = nc.dram_tensor("x", [PB, 128, 1024], F16, kind="ExternalInput")
    yd = nc.dram_tensor("y", [PB, 128, NT, 128], F8, kind="ExternalInput")
    zd = nc.dram_tensor("z", [PB, 128, 1048], F8, kind="ExternalInput")
    wpd = nc.dram_tensor("wp", [128, 2048], F8, kind="ExternalInput")
    od = nc.dram_tensor("out", [PB, 128, 1024], F16, kind="ExternalOutput")

    with tile.TileContext(nc) as tc:
        with (
            tc.tile_pool(name="sb", bufs=1) as sb,
            tc.tile_pool(name="ps", bufs=1, space=bass.MemorySpace.PSUM) as ps,
        ):
            # ---- persistent SBUF tiles -------------------------------------
            xin = sb.tile([128, PB, 1024], F16, tag="xin")
            xot = sb.tile([128, PB, 1024], F16, tag="xot")
            yin = sb.tile([128, PB, NT, 128], F8, tag="yin")
            zin = sb.tile([128, PB, 1048], F8, tag="zin")
            wpt = sb.tile([128, 2048], F8, tag="wpt")
            lens = sb.tile([128, 6], F32, tag="lens")   # lx0 lx1 ly0 ly1 lz0 lz1
            alp = sb.tile([128, 4], F32, tag="alp")     # 1/len for c cols
            rec16 = sb.tile([128, 4], F32, tag="rec16")  # 16/len for c cols
            iot = sb.tile([128, 1024], F16, tag="iot")  # t index
            mxt = sb.tile([128, PB, 1024], F16, tag="mxt")  # x masks
            ones8 = sb.tile([128, 1], F8, tag="ones8")
            ct = sb.tile([128, 4], F8, tag="ct")        # c1b0 c1b1 c2b0 c2b1
            ht = sb.tile([128, 4, 2, 2], F8, tag="ht")  # [net, k, b]
            sbv = sb.tile([128, 2, 2], F32, tag="sbv")  # [s|bias, b]
            cls10 = sb.tile([128, 2], F32, tag="cls10")  # (1, 0)
            gdum = sb.tile([1, 2], F32, tag="gdum")
            # ---- PSUM ------------------------------------------------------
            c_ps = ps.tile([128, 4], F32, tag="c_ps")
            h_ps = ps.tile([128, 4, 2, 2], F32, tag="h_ps")
            o2_ps = [[ps.tile([128, 1], F32, tag=f"o2_{c}{b}", name=f"o2_{c}{b}")
                      for b in range(2)] for c in range(2)]  # [cls][b]

            ints = zin[:, 0, 1024:1048].bitcast(I32)   # [128, 6] int32 view

            # ---- DMA issues (program order per engine = issue order) -------
            # ACT's SEQ is blocked ~1.3us by the auto-inserted gelu-table
            # load and its DGE is slow, so ACT issues no loads.
            # SP ring: z(+lens), y, x0 + stores. Pool: wp, z1?, x1 + store.
            nc.sync.dma_start(out=zin[:, 0:1, :],
                              in_=zd[:].rearrange("b p c -> p b c")[:, 0:1])
            nc.gpsimd.dma_start(out=wpt[:], in_=wpd[:])
            nc.sync.dma_start(out=yin[:],
                              in_=yd[:].rearrange("b p n d -> p b n d"))
            nc.gpsimd.dma_start(out=zin[:, 1:2, :],
                                in_=zd[:].rearrange("b p c -> p b c")[:, 1:2])
            nc.sync.dma_start(out=xin[:, 0:1, :], in_=xd[:].rearrange("b p t -> p b t")[:, 0:1])
            nc.gpsimd.dma_start(out=xin[:, 1:2, :], in_=xd[:].rearrange("b p t -> p b t")[:, 1:2])
            # gelu table preload via dummy activation (ACT engine)
            nc.vector.memset(gdum[0:1, 0:1], 0.0)
            nc.scalar.activation(gdum[0:1, 1:2], gdum[0:1, 0:1], act)

            # ---- early constants (no input deps) ---------------------------
            nc.vector.memset(ones8[:], 1.0)
            nc.vector.memset(cls10[:, 0:1], 1.0)
            nc.vector.memset(cls10[:, 1:2], 0.0)
            nc.gpsimd.iota(iot[:], pattern=[[1, 1024]], base=0, channel_multiplier=0,
                           allow_small_or_imprecise_dtypes=True)

            # ---- lens-derived (dep: wp part A) -----------------------------
            nc.vector.tensor_copy(lens[:], ints[:])
            nc.vector.reciprocal(alp[:], lens[:, 2:6])
            nc.vector.tensor_scalar(rec16[:], alp[:], 16.0, None, OP.mult)
            # x masks (dep: lens + iota); b0 on Pool, b1 on DVE
            nc.gpsimd.tensor_scalar(mxt[:, 0, :], iot[:], lens[:, 0:1], None, OP.is_lt)
            nc.vector.tensor_scalar(mxt[:, 1, :], iot[:], lens[:, 1:2], None, OP.is_lt)

            # ---- per-sample MLP pipelines ----------------------------------
            # c cols: 0 = c1b0 (y0), 1 = c1b1 (y1), 2 = c2b0 (z0), 3 = c2b1 (z1)
            wl1 = wpt[:, 0:1024]
            wl2 = wpt[:, 1024:2048]
            yinf = yin[:].rearrange("p b n d -> p b (n d)")

            def csum(tens, b, col):
                for j in range(NT):
                    nc.tensor.matmul(c_ps[:, col:col + 1],
                                     lhsT=tens[:, b, j * 128:(j + 1) * 128],
                                     rhs=ones8[:], start=(j == 0), stop=(j == NT - 1))

            def ctpair(c0):
                # ct = c_raw * 16/len (the 1/16 is repaid in the gelu scale)
                nc.vector.tensor_tensor(ct[:, c0:c0 + 2], c_ps[:, c0:c0 + 2],
                                        rec16[:, c0:c0 + 2], OP.mult)

            def l1(nets, b, col):
                for n in nets:
                    for k in range(2):
                        nc.tensor.matmul(h_ps[:, n, k, b:b + 1], rhs=ct[:, col:col + 1],
                                         lhsT=wl1[:, n * 256 + k * 128:n * 256 + (k + 1) * 128],
                                         start=True, stop=True)

            def gelu(b):
                # all 4 nets of sample b in one ACT op (scale repays the x16)
                nc.scalar.activation(ht[:, :, :, b:b + 1], h_ps[:, :, :, b:b + 1],
                                     act, scale=1.0 / 16.0)

            def l2(nets, b, start, stop):
                for i, n in enumerate(nets):
                    for k in range(2):
                        cls = n % 2
                        nc.tensor.matmul(o2_ps[cls][b][:], rhs=ht[:, n, k, b:b + 1],
                                         lhsT=wl2[:, n * 256 + k * 128:n * 256 + (k + 1) * 128],
                                         start=(start and k == 0),
                                         stop=(stop and k == 1))

            # c sums in expected DMA-arrival order (PE executes in order)
            csum(zin, 0, 2)
            csum(zin, 1, 3)
            ctpair(2)
            csum(yinf, 0, 0)
            csum(yinf, 1, 1)
            ctpair(0)
            for b in range(PB):
                l1((2, 3), b, 2 + b)
                l1((0, 1), b, b)
                gelu(b)
                l2((2, 3), b, start=True, stop=False)
                l2((0, 1), b, start=False, stop=True)

            # ---- finalize s/bias and elementwise ---------------------------
            # xm = x * mask precomputed off the s-path (tensor_tensor);
            # the s-gated ops are then 2x-mode tensor_scalar multiplies.
            o_ap = od[:].rearrange("b p t -> p b t")
            xm = sb.tile([128, PB, 512], F16, tag="xm")
            # masked-half premultiplies: b0 on DVE, b1 on Pool
            nc.vector.tensor_tensor(xm[:, 0, :], xin[:, 0, 512:1024],
                                    mxt[:, 0, 512:1024], OP.mult)
            nc.gpsimd.tensor_tensor(xm[:, 1, :], xin[:, 1, 512:1024],
                                    mxt[:, 1, 512:1024], OP.mult)
            for b in range(PB):
                # sbv[:, 0, b] = 1 + o2/2 (s) ; sbv[:, 1, b] = o2/2 (bias)
                nc.vector.scalar_tensor_tensor(sbv[:, 0, b:b + 1], o2_ps[0][b][:], 0.5,
                                               cls10[:, 0:1], OP.mult, OP.add)
                nc.vector.tensor_scalar(sbv[:, 1, b:b + 1], o2_ps[1][b][:], 0.5,
                                        None, OP.mult)
            for b in range(PB):
                # col 0 with bias (tiny, DVE)
                nc.vector.scalar_tensor_tensor(xot[:, b, 0:1], xin[:, b, 0:1],
                                               sbv[:, 0, b:b + 1], sbv[:, 1, b:b + 1],
                                               OP.mult, OP.add)
                # masked half [512:1024) on DVE (4x tensor_scalar on xm)
                nc.vector.tensor_scalar(xot[:, b, 512:1024], xm[:, b, :],
                                        sbv[:, 0, b:b + 1], None, OP.mult)
            # unmasked halves [1:512): b0 on ACT, b1 on Pool
            nc.scalar.mul(xot[:, 0, 1:512], xin[:, 0, 1:512], sbv[:, 0, 0:1])
            nc.gpsimd.tensor_scalar(xot[:, 1, 1:512], xin[:, 1, 1:512],
                                    sbv[:, 0, 1:2], None, OP.mult)
            # stores: masked halves first (ready first)
            nc.sync.dma_start(out=o_ap[:, 0, 512:1024], in_=xot[:, 0, 512:1024])
            nc.gpsimd.dma_start(out=o_ap[:, 1, 512:1024], in_=xot[:, 1, 512:1024])
            nc.scalar.dma_start(out=o_ap[:, 0, 0:512], in_=xot[:, 0, 0:512])
            nc.sync.dma_start(out=o_ap[:, 1, 0:512], in_=xot[:, 1, 0:512])

    nc.compile()
    return nc


_NC_CACHE = None


def _get_nc():
    global _NC_CACHE
    if _NC_CACHE is None:
        _NC_CACHE = build_nc()
    return _NC_CACHE


def _pack_params(arr):
    wp = np.empty((128, 2048), NP8)
    for n_i, n in enumerate(NETS):
        wp[:, n_i * 256:(n_i + 1) * 256] = arr[f"{n}_l1_w"].astype(NP8)
        # wl2[p, k*128 + d] = l2_w[k*128 + p, d]
        w2 = arr[f"{n}_l2_w"].reshape(2, 128, 128).transpose(1, 0, 2).reshape(128, 256)
        wp[:, 1024 + n_i * 256:1024 + (n_i + 1) * 256] = w2.astype(NP8)
    return np.ascontiguousarray(wp)


def _shuffle(a):
    # [pb, L, D] -> [pb, p, n, d] with t = n*128 + p (partition-contiguous)
    pb = a.shape[0]
    return np.ascontiguousarray(
        a.reshape(pb, NT, 128, 128).transpose(0, 2, 1, 3))


def _make_in_maps(inputs):
    arr = {k: np.ascontiguousarray(np.asarray(v),
                                   dtype=(np.int32 if k.startswith("len") else np.float32))
           for k, v in inputs.items()}
    xs = np.ascontiguousarray(arr["x"].transpose(0, 2, 1).astype(np.float16))
    ys = _shuffle(arr["y"]).astype(NP8)
    zs = _shuffle(arr["z"]).astype(NP8).reshape(B, 128, 1024)
    wp = _pack_params(arr)
    in_maps = []
    for c in range(NCORES):
        sl = slice(c * PB, (c + 1) * PB)
        lx, ly, lz = arr["len_x"][sl], arr["len_y"][sl], arr["len_z"][sl]
        lens6 = np.array([lx[0], lx[1], ly[0], ly[1], lz[0], lz[1]], np.int32)
        lb = np.frombuffer(lens6.tobytes(), dtype=np.uint8).view(NP8)
        zl = np.empty((PB, 128, 1048), NP8)
        zl[:, :, 0:1024] = zs[sl]
        zl[:, :, 1024:1048] = lb[None, None, :]
        in_maps.append({
            "x": np.ascontiguousarray(xs[sl]),
            "y": np.ascontiguousarray(ys[sl]),
            "z": zl,
            "wp": wp,
        })
    return in_maps


def run(inputs, trace=False, **kw):
    """Run on the 8 NeuronCores; returns (out [16,1024,128] f32, results)."""
    from concourse.bass_utils import run_bass_kernel_spmd

    nc = _get_nc()
    in_maps = _make_in_maps(inputs)
    res = run_bass_kernel_spmd(nc, in_maps, core_ids=list(range(NCORES)),
                               trace=trace, **kw)
    out = np.concatenate(
        [np.asarray(r["out"]).transpose(0, 2, 1) for r in res.results], axis=0)
    return out.astype(np.float32), res


def kernel(**inputs):
    out, _ = run(inputs, trace=False)
    return out

